# revision 1
# baseline (speedup 1.0000x reference)
"""Causal multi-head attention (B=4, T=2048, C=1024, 16 heads) on 8 TRN2 NeuronCores.

Sharding: data-parallel over (batch, q-chunk-pair). Core 2*b+h handles batch b
and two 512-row q-chunks chosen so every core runs an identical program:
  core (b,0): chunk A = rows [0:512]     (program kv extent 1024)
              chunk B = rows [1536:2048] (program kv extent 2048)
  core (b,1): chunk A = rows [512:1024]  (kv extent 1024)
              chunk B = rows [1024:1536] (kv extent 2048, data extent 1536)
Causality inside the rectangles is enforced with per-core {0,1} multiplicative
masks supplied as data, so the instruction stream is core-independent (SPMD).

Everything on-device lives transposed ([feature, token]): softmax denominators
come out of the TensorEngine via a ones-column appended to V, and no on-device
transposes are needed; the host transposes x in and the output out.

Inputs/weights/activations are bf16 (PE at full rate, fp32 PSUM accumulation);
the l/normalization path is fp32. Score matmuls for a head pair run on PE
row-groups 0-63 / 64-127 concurrently (contract dim is 64).

The emission order interleaves PE-heavy projection work into the ACT-bound
attention phases: K/V projections for kv [1024:2048] and the chunk-B Q
projection are spread between chunk-A head pairs; the chunk-A output
projection is spread between chunk-B head pairs.
"""

import numpy as np
import ml_dtypes

B, T, C, NH, D = 4, 2048, 1024, 16, 64
P = 128
CH = 512                # q-chunk size
KV_EXT = (1024, 2048)   # program kv extent for chunk A / chunk B

_CACHE = {}


def _build():
    import concourse.bacc as bacc
    import concourse.tile as tile
    import concourse.mybir as mybir
    from concourse.bass import ts, ds

    f32 = mybir.dt.float32
    bf16 = mybir.dt.bfloat16
    ID = mybir.ActivationFunctionType.Identity
    EXP = mybir.ActivationFunctionType.Exp
    COPY = mybir.ActivationFunctionType.Copy
    MUL = mybir.AluOpType.mult
    ADD = mybir.AluOpType.add

    nc = bacc.Bacc("TRN2", target_bir_lowering=False, debug=False, num_devices=8)

    def din(name, shape, dt=bf16):
        return nc.dram_tensor(name, list(shape), dt, kind="ExternalInput").ap()

    xqT = din("xqT", (C, 2 * CH))    # x^T, this core's q rows (A then B)
    xkvT = din("xkvT", (C, T))       # x^T, full batch (for K/V)
    wqT = din("wqT", (C, C))         # (Wq/8)^T
    wkT = din("wkT", (C, C))
    wvT = din("wvT", (C, C))
    woT = din("woT", (C, C))
    bq = din("bq", (P, C // P), f32)     # bq/8, chunked [128, 8]
    bk = din("bk", (P, C // P), f32)
    bo = din("bo", (P, C // P), f32)
    maskA = din("maskA", (KV_EXT[0], CH))     # {0,1}, [kv, q] chunk A
    maskB = din("maskB", (1024, CH))          # chunk B, kv in [1024:2048]
    out = nc.dram_tensor("out", [C, 2 * CH], f32, kind="ExternalOutput").ap()

    KC = C // P        # 8 contraction chunks
    NT = T // P        # 16 kv chunks of the full batch

    wq_v = wqT.rearrange("(ko p) m -> p ko m", p=P)
    wk_v = wkT.rearrange("(ko p) m -> p ko m", p=P)
    wo_v = woT.rearrange("(ko p) m -> p ko m", p=P)
    wv_v = wvT.rearrange("(ko p) c -> p ko c", p=P)
    xkv_v = xkvT.rearrange("(ko p) t -> p ko t", p=P)
    xq_v = xqT.rearrange("(ko p) t -> p ko t", p=P)
    maskA_v = maskA.rearrange("(ko p) q -> p ko q", p=P)
    maskB_v = maskB.rearrange("(ko p) q -> p ko q", p=P)

    from contextlib import ExitStack
    with ExitStack() as ctx:
        tc = ctx.enter_context(tile.TileContext(nc))

        consts = ctx.enter_context(tc.tile_pool(name="consts", bufs=1))
        big = ctx.enter_context(tc.tile_pool(name="big", bufs=1))
        wpool = ctx.enter_context(tc.tile_pool(name="w", bufs=2))
        xkpool = ctx.enter_context(tc.tile_pool(name="xk", bufs=2))
        xvpool = ctx.enter_context(tc.tile_pool(name="xv", bufs=2))
        qpool = ctx.enter_context(tc.tile_pool(name="q", bufs=1))
        mpool = ctx.enter_context(tc.tile_pool(name="m", bufs=1))
        xqpool = ctx.enter_context(tc.tile_pool(name="xq", bufs=1))
        ptpool = ctx.enter_context(tc.tile_pool(name="pt", bufs=4))
        ctxpool = ctx.enter_context(tc.tile_pool(name="ctx", bufs=1))
        lpool = ctx.enter_context(tc.tile_pool(name="l", bufs=2))
        l0pool = ctx.enter_context(tc.tile_pool(name="l0", bufs=2))
        lbpool = ctx.enter_context(tc.tile_pool(name="lb", bufs=2))
        cspool = ctx.enter_context(tc.tile_pool(name="cs", bufs=3))
        opool = ctx.enter_context(tc.tile_pool(name="o", bufs=2))
        psumP = ctx.enter_context(tc.tile_pool(name="psumP", bufs=2, space="PSUM"))
        psumS = ctx.enter_context(tc.tile_pool(name="psumS", bufs=2, space="PSUM"))
        psumX = ctx.enter_context(tc.tile_pool(name="psumX", bufs=2, space="PSUM"))

        bq_sb = consts.tile([P, KC], f32)
        bk_sb = consts.tile([P, KC], f32)
        bo_sb = consts.tile([P, KC], f32)
        nc.sync.dma_start(bq_sb[:], bq)
        nc.sync.dma_start(bk_sb[:], bk)
        nc.sync.dma_start(bo_sb[:], bo)

        KT_sb = big.tile([P, KC, T], bf16)          # K^T  [c, t]
        V_sb = big.tile([P, NT, NH, D + 1], bf16)   # V + ones col per chunk/head
        nc.vector.memset(V_sb[:, :, :, D : D + 1], 1.0)
        wvt0 = big.tile([P, KC, CH], bf16)          # Wv^T halves, resident
        wvt1 = big.tile([P, KC, CH], bf16)
        nc.sync.dma_start(wvt0[:], wv_v[:, :, 0:CH])
        nc.sync.dma_start(wvt1[:], wv_v[:, :, CH:C])
        wvt = [wvt0, wvt1]

        # ---------- emission helpers ----------
        XK = {}

        def kt_proj(ft, m0, m1):
            """KT[:, m0:m1, 512*ft:...] from a cached xk tile."""
            if ft not in XK:
                XK[ft] = xkpool.tile([P, KC, 512], bf16, tag="xk",
                                     name=f"xk{ft}")
                nc.sync.dma_start(XK[ft][:], xkv_v[:, :, ds(512 * ft, 512)])
            xk = XK[ft]
            for m in range(m0, m1):
                wt = wpool.tile([P, KC, P], bf16, tag="w", name=f"wk{ft}{m}")
                nc.sync.dma_start(wt[:], wk_v[:, :, ts(m, P)])
                ps = psumP.tile([P, 512], f32, tag="psP", name=f"pk{ft}{m}")
                for k in range(KC):
                    nc.tensor.matmul(ps[:], wt[:, k, :], xk[:, k, :],
                                     start=(k == 0), stop=(k == KC - 1))
                nc.scalar.activation(KT_sb[:, m, ds(512 * ft, 512)], ps[:],
                                     ID, bias=bk_sb[:, m : m + 1])

        def v_proj(i):
            """V rows [128*i : 128*(i+1)], all channels."""
            xv = xvpool.tile([P, KC, P], bf16, tag="xv", name=f"xv{i}")
            nc.sync.dma_start(xv[:], xkv_v[:, :, ts(i, P)])
            for chh in range(2):
                ps = psumP.tile([P, 512], f32, tag="psP", name=f"pv{i}{chh}")
                for k in range(KC):
                    nc.tensor.matmul(ps[:], xv[:, k, :], wvt[chh][:, k, :],
                                     start=(k == 0), stop=(k == KC - 1))
                nc.scalar.activation(
                    V_sb[:, i, ds(8 * chh, 8), 0:D],
                    ps.rearrange("p (h d) -> p h d", d=D), COPY)

        QT = {}

        def q_proj(qc, m0, m1):
            if qc not in QT:
                QT[qc] = qpool.tile([P, KC, CH], bf16, tag=f"qt{qc}",
                                    name=f"qt{qc}")
            if ("xq", qc) not in QT:
                QT[("xq", qc)] = xqpool.tile([P, KC, CH], bf16, tag="xq",
                                             name=f"xq{qc}")
                nc.sync.dma_start(QT[("xq", qc)][:],
                                  xq_v[:, :, ds(CH * qc, CH)])
            xq = QT[("xq", qc)]
            for m in range(m0, m1):
                wt = wpool.tile([P, KC, P], bf16, tag="w", name=f"wq{qc}{m}")
                nc.sync.dma_start(wt[:], wq_v[:, :, ts(m, P)])
                ps = psumP.tile([P, CH], f32, tag="psP", name=f"pq{qc}{m}")
                for k in range(KC):
                    nc.tensor.matmul(ps[:], wt[:, k, :], xq[:, k, :],
                                     start=(k == 0), stop=(k == KC - 1))
                nc.scalar.activation(QT[qc][:, m, :], ps[:], ID,
                                     bias=bq_sb[:, m : m + 1])

        def o_proj(qc, ctxT, m):
            wt = wpool.tile([P, KC, P], bf16, tag="w", name=f"wo{qc}{m}")
            nc.sync.dma_start(wt[:], wo_v[:, :, ts(m, P)])
            ps = psumP.tile([P, CH], f32, tag="psP", name=f"po{qc}{m}")
            for k in range(KC):
                nc.tensor.matmul(ps[:], wt[:, k, :], ctxT[:, k, :],
                                 start=(k == 0), stop=(k == KC - 1))
            o_sb = opool.tile([P, CH], f32, tag="o", name=f"o{qc}{m}")
            nc.scalar.activation(o_sb[:], ps[:], ID, bias=bo_sb[:, m : m + 1])
            nc.sync.dma_start(out[ts(m, P), ds(CH * qc, CH)], o_sb[:])

        def attn_pair(qc, hp, msk, ctxT):
            E = KV_EXT[qc]
            NKV = E // P
            ctx_ps = [psumX.tile([P, CH], f32, tag="psX", name=f"psX{qc}{hp}{i}")
                      for i in range(2)]
            for kvc in range(NKV):
                st = psumS.tile([P, 2, CH], f32, tag="psS",
                                name=f"psS{qc}{hp}{kvc}")
                for hh in range(2):
                    # contract dim 64 at PE row-group 64*hh: the two heads'
                    # score matmuls run concurrently in the array
                    nc.tensor.matmul(
                        st[:, hh, :],
                        KT_sb[ds(64 * hh, 64), hp, ds(P * kvc, P)],
                        QT[qc][ds(64 * hh, 64), hp, :],
                        start=True, stop=True)
                pt = ptpool.tile([P, 2, CH], bf16, tag="pt",
                                 name=f"pt{qc}{hp}{kvc}")
                nc.scalar.activation(pt[:], st[:], EXP)
                mi = kvc if qc == 0 else kvc - NKV // 2
                if mi >= 0:   # causal mask (chunk A: all; chunk B: kv >= 1024)
                    nc.vector.tensor_tensor(
                        pt[:], pt[:],
                        msk[:, mi : mi + 1, :].to_broadcast((P, 2, CH)), MUL)
                for hh in range(2):
                    nc.tensor.matmul(
                        ctx_ps[hh][0 : D + 1, :],
                        V_sb[:, kvc, 2 * hp + hh, :],
                        pt[:, hh, :],
                        start=(kvc == 0), stop=(kvc == NKV - 1))
            # Epilogue. Free the PSUM banks fast (reciprocal of row D + DVE
            # copy of rows [0:D) to SBUF); the 1/l row is hopped to physical
            # partition 0 (the only one HW partition_broadcast reads) on the
            # GpSimd DMA queue, broadcast on GpSimd, normalized on DVE, and
            # partition-remapped into ctxT with a GpSimd-queued DMA.
            cs = []
            for hh in range(2):
                l_sb = lpool.tile([P, CH], f32, tag="l", name=f"l{qc}{hp}{hh}")
                nc.vector.reciprocal(l_sb[D : D + 1, :],
                                     ctx_ps[hh][D : D + 1, :])
                l0 = l0pool.tile([1, CH], f32, tag="l0", name=f"l0{qc}{hp}{hh}")
                nc.gpsimd.dma_start(l0[:], l_sb[D : D + 1, :])
                c_scr = cspool.tile([P, CH], f32, tag="cs",
                                    name=f"cs{qc}{hp}{hh}")
                nc.vector.tensor_copy(c_scr[0:D, :], ctx_ps[hh][0:D, :])
                cs.append((l0, c_scr))
            for hh in range(2):
                l0, c_scr = cs[hh]
                linv = lbpool.tile([P, CH], f32, tag="lb", name=f"lb{qc}{hp}{hh}")
                nc.gpsimd.partition_broadcast(linv[0:D, :], l0[:], channels=D)
                if hh == 0:
                    nc.vector.tensor_tensor(ctxT[0:D, hp, :], c_scr[0:D, :],
                                            linv[0:D, :], MUL)
                else:
                    c2 = cspool.tile([P, CH], bf16, tag="cs2",
                                     name=f"cs2{qc}{hp}")
                    nc.vector.tensor_tensor(c2[0:D, :], c_scr[0:D, :],
                                            linv[0:D, :], MUL)
                    nc.gpsimd.dma_start(ctxT[ds(64, 64), hp, :], c2[0:D, :])

        # ---------- emission schedule ----------
        # prologue: K/V for kv [0:1024], Q for chunk A
        for ft in range(2):
            kt_proj(ft, 0, 4); kt_proj(ft, 4, 8)
        for i in range(8):
            v_proj(i)
        q_proj(0, 0, 4); q_proj(0, 4, 8)

        mskA = mpool.tile([P, KC, CH], bf16, tag="mask", name="mA")
        nc.sync.dma_start(mskA[:], maskA_v)

        # chunk A attention, with kv[1024:2048] K/V projections and the
        # chunk-B Q projection interleaved as PE filler
        ctxT_A = ctxpool.tile([P, KC, CH], bf16, tag="ctxA", name="ctxA")
        fillers = ([lambda ft=ft, m0=m0: kt_proj(ft, m0, m0 + 4)
                    for ft in (2, 3) for m0 in (0, 4)]
                   + [lambda i=i: v_proj(i) for i in range(8, 16)]
                   + [lambda m0=m0: q_proj(1, m0, m0 + 4) for m0 in (0, 4)])
        fi = 0
        for hp in range(NH // 2):
            attn_pair(0, hp, mskA, ctxT_A)
            take = (len(fillers) - fi + (NH // 2 - hp) - 1) // (NH // 2 - hp)
            for _ in range(take):
                if fi < len(fillers):
                    fillers[fi](); fi += 1
        while fi < len(fillers):
            fillers[fi](); fi += 1

        # chunk B attention, with chunk-A output projection interleaved
        mskB = mpool.tile([P, KC, CH], bf16, tag="mask", name="mB")
        nc.sync.dma_start(mskB[:], maskB_v)
        ctxT_B = ctxpool.tile([P, KC, CH], bf16, tag="ctxB", name="ctxB")
        for hp in range(NH // 2):
            attn_pair(1, hp, mskB, ctxT_B)
            o_proj(0, ctxT_A, hp)
        for m in range(NH // 2, KC):
            o_proj(0, ctxT_A, m)
        for m in range(KC):
            o_proj(1, ctxT_B, m)

    nc.compile()
    return nc


def _shard_inputs(x, Wq, bq, bk_, bv, bo, WqT, WkT, WvT, WoT):
    """Build the 8 per-core input maps (bf16 data tensors, fp32 biases).

    bv is folded into the output-projection bias: ctx = ctx0 + 1*bv^T, so
    out = ctx0 @ Wo^T + (bo + Wo @ bv)."""
    bf = ml_dtypes.bfloat16
    in_maps = []
    rows = {0: (np.arange(0, 512), np.arange(1536, 2048)),
            1: (np.arange(512, 1024), np.arange(1024, 1536))}
    kv = np.arange(T)
    bq8 = np.ascontiguousarray((bq / 8.0).reshape(C // P, P).T)
    bk8 = np.ascontiguousarray(bk_.reshape(C // P, P).T)
    bo_f = bo + WoT.T @ bv
    bo8 = np.ascontiguousarray(bo_f.reshape(C // P, P).T)
    wq16, wk16 = WqT.astype(bf), WkT.astype(bf)
    wv16, wo16 = WvT.astype(bf), WoT.astype(bf)
    for b in range(B):
        xT = np.ascontiguousarray(x[b].T).astype(bf)     # (C, T)
        for h in range(2):
            qA, qB = rows[h]
            xqT = np.ascontiguousarray(xT[:, np.concatenate([qA, qB])])
            mA = (kv[:1024, None] <= qA[None, :]).astype(bf)
            mB = (kv[1024:, None] <= qB[None, :]).astype(bf)
            in_maps.append({
                "xqT": xqT, "xkvT": xT,
                "wqT": wq16, "wkT": wk16, "wvT": wv16, "woT": wo16,
                "bq": bq8, "bk": bk8, "bo": bo8,
                "maskA": np.ascontiguousarray(mA),
                "maskB": np.ascontiguousarray(mB),
            })
    return in_maps


def kernel(x, Wq, bq, Wk, bk, Wv, bv, Wo, bo):
    from concourse.bass_utils import run_bass_kernel_spmd

    x = np.asarray(x, np.float32)
    Wq = np.asarray(Wq, np.float32); bq = np.asarray(bq, np.float32)
    Wk = np.asarray(Wk, np.float32); bk = np.asarray(bk, np.float32)
    Wv = np.asarray(Wv, np.float32); bv = np.asarray(bv, np.float32)
    Wo = np.asarray(Wo, np.float32); bo = np.asarray(bo, np.float32)

    if "nc" not in _CACHE:
        _CACHE["nc"] = _build()
    nc = _CACHE["nc"]

    WqT = np.ascontiguousarray(Wq.T / 8.0)
    WkT = np.ascontiguousarray(Wk.T)
    WvT = np.ascontiguousarray(Wv.T)
    WoT = np.ascontiguousarray(Wo.T)
    in_maps = _shard_inputs(x, Wq, bq, bk, bv, bo, WqT, WkT, WvT, WoT)

    res = run_bass_kernel_spmd(nc, in_maps, core_ids=list(range(8)))
    outf = np.empty((B, T, C), np.float32)
    rows = {0: (np.arange(0, 512), np.arange(1536, 2048)),
            1: (np.arange(512, 1024), np.arange(1024, 1536))}
    for b in range(B):
        for h in range(2):
            o = res.results[2 * b + h]["out"]          # (C, 1024) transposed
            qA, qB = rows[h]
            outf[b, qA, :] = o[:, :512].T
            outf[b, qB, :] = o[:, 512:].T
    return outf



# revision 4
# speedup vs baseline: 1.8910x; 1.8910x over previous
"""Causal multi-head attention (B=4, T=2048, C=1024, 16 heads) on 8 TRN2 NeuronCores.

Sharding: core (b, g) handles batch b and head-group g (8 heads = 512 features).
Each core projects Q/K/V for its own heads only (no duplicated projection work),
runs causal attention for its 8 heads over the full sequence, and computes a
PARTIAL output projection (contraction over its 512 ctx features). The host sums
the two partials per batch and adds the (bv-folded) output bias.

Causality: q-chunks of 512 attend to kv in [0, 512(j+1)); within the diagonal
512x512 block, score/exp/AV work is trimmed at 128 granularity and the
remaining triangular 128x128 blocks are masked with a single {0,1} tile.

Engine budget per core (bf16 matmuls, fp32 PSUM):
  PE  ~180us: QKVO projections (112) + scores/AV head-pair packed (60) + l-sums
  ACT ~163us: exp only (projection epilogues are on DVE instead)
  DVE ~110us: bias epilogues, P accumulation (for softmax denominators),
              masks, 1/l, ctx normalize, PSUM->SBUF copies
Softmax denominators: P tiles are accumulated on DVE into acc[kv,2,q]; a
ones-vector matmul reduces over kv partitions; normalize multiplies ctx by a
gpsimd-broadcast 1/l tile before the output projection.

Emission interleaves projection tiles as PE filler into the ACT-bound
attention stream (scores pipelined one step ahead of AV).
"""

import numpy as np
import ml_dtypes

B, T, C, NH, D = 4, 2048, 1024, 16, 64
P = 128
G = 8                 # heads per core
CH = 512              # q-chunk size
NCH = T // CH         # 4 q-chunks
KC = C // P           # 8 contraction chunks for QKV projections
OC = (C // 2) // P    # 4 contraction chunks for the partial O projection
NHP = G // 2          # 4 head pairs per core

_CACHE = {}


def _build():
    import concourse.bacc as bacc
    import concourse.tile as tile
    import concourse.mybir as mybir
    from concourse.bass import ts, ds

    f32 = mybir.dt.float32
    bf16 = mybir.dt.bfloat16
    EXP = mybir.ActivationFunctionType.Exp
    MUL = mybir.AluOpType.mult
    ADD = mybir.AluOpType.add

    nc = bacc.Bacc("TRN2", target_bir_lowering=False, debug=False, num_devices=8)

    def din(name, shape, dt=bf16):
        return nc.dram_tensor(name, list(shape), dt, kind="ExternalInput").ap()

    xT = din("xT", (C, T))           # x^T for this batch
    wqT = din("wqT", (C, CH))        # (Wq/8)^T columns for this head group
    wkT = din("wkT", (C, CH))
    wvT = din("wvT", (C, CH))
    woT = din("woT", (CH, C))        # Wo^T rows for this head group
    bq = din("bq", (P, NHP), f32)    # bq/8, chunked per 128-feature block
    bk = din("bk", (P, NHP), f32)
    tri = din("tri", (P, P))         # {0,1}, tri[kv, q] = kv <= q
    out = nc.dram_tensor("out", [C, T], f32, kind="ExternalOutput").ap()

    x_v = xT.rearrange("(k p) t -> p k t", p=P)      # [128, 8, 2048]
    wq_v = wqT.rearrange("(k p) m -> p k m", p=P)    # [128, 8, 512]
    wk_v = wkT.rearrange("(k p) m -> p k m", p=P)
    wv_v = wvT.rearrange("(k p) m -> p k m", p=P)
    wo_v = woT.rearrange("(k p) m -> p k m", p=P)    # [128, 4, 1024]

    from contextlib import ExitStack
    with ExitStack() as ctx:
        tc = ctx.enter_context(tile.TileContext(nc))

        consts = ctx.enter_context(tc.tile_pool(name="consts", bufs=1))
        big = ctx.enter_context(tc.tile_pool(name="big", bufs=1))
        ctxpool = ctx.enter_context(tc.tile_pool(name="ctxT", bufs=2))
        ptpool = ctx.enter_context(tc.tile_pool(name="pt", bufs=3))
        accpool = ctx.enter_context(tc.tile_pool(name="acc", bufs=2))
        lrpool = ctx.enter_context(tc.tile_pool(name="lr", bufs=2))
        lbpool = ctx.enter_context(tc.tile_pool(name="lb", bufs=2))
        opool = ctx.enter_context(tc.tile_pool(name="o", bufs=2))
        psumS = ctx.enter_context(tc.tile_pool(name="psumS", bufs=2, space="PSUM"))
        psumC = ctx.enter_context(tc.tile_pool(name="psumC", bufs=2, space="PSUM"))
        psumP = ctx.enter_context(tc.tile_pool(name="psumP", bufs=2, space="PSUM"))

        bq_sb = consts.tile([P, NHP], f32)
        bk_sb = consts.tile([P, NHP], f32)
        tri_sb = consts.tile([P, 1, P], bf16)
        ones_sb = consts.tile([P, 1], bf16)
        warm = consts.tile([1, 2], f32)
        nc.sync.dma_start(bq_sb[:], bq)
        nc.sync.dma_start(bk_sb[:], bk)
        nc.sync.dma_start(tri_sb[:, 0, :], tri)
        nc.vector.memset(ones_sb[:], 1.0)
        nc.vector.memset(warm[:], 0.0)
        # preload the exp table set early
        nc.scalar.activation(warm[:], warm[:], EXP)

        X = big.tile([P, KC, T], bf16)
        WQ = big.tile([P, KC, CH], bf16)
        WK = big.tile([P, KC, CH], bf16)
        WV = big.tile([P, KC, CH], bf16)
        WO = big.tile([P, OC, C], bf16)
        KT = big.tile([P, NHP, T], bf16)    # K^T  [d(2-head packed), hp, t]
        QT = big.tile([P, NHP, T], bf16)
        V = big.tile([P, T // P, G, D], bf16)  # [kv_local, kv_chunk, head, d]

        # DMA order = first-use order: X slice 0, WK, WV, WQ, rest of X, WO
        nc.sync.dma_start(X[:, :, 0:CH], x_v[:, :, 0:CH])
        nc.sync.dma_start(WK[:], wk_v)
        nc.sync.dma_start(WV[:], wv_v)
        nc.sync.dma_start(WQ[:], wq_v)
        for tb in range(1, NCH):
            nc.sync.dma_start(X[:, :, ds(CH * tb, CH)], x_v[:, :, ds(CH * tb, CH)])
        nc.sync.dma_start(WO[:], wo_v)

        # ---------- projection tile emitters (filler units) ----------
        def kt_tile(tb, hp):
            """KT[:, hp, 512*tb : ...] <- (Wk chunk)^T @ x chunk + bk."""
            ps = psumP.tile([P, CH], f32, tag="pp", name=f"pk{tb}{hp}")
            for k in range(KC):
                nc.tensor.matmul(ps[:], WK[:, k, ts(hp, P)],
                                 X[:, k, ds(CH * tb, CH)],
                                 start=(k == 0), stop=(k == KC - 1))
            nc.vector.tensor_scalar(
                out=KT[:, hp, ds(CH * tb, CH)], in0=ps[:],
                scalar1=bk_sb[:, hp : hp + 1], scalar2=None, op0=ADD)

        def q_tile(j, hp):
            ps = psumP.tile([P, CH], f32, tag="pp", name=f"pq{j}{hp}")
            for k in range(KC):
                nc.tensor.matmul(ps[:], WQ[:, k, ts(hp, P)],
                                 X[:, k, ds(CH * j, CH)],
                                 start=(k == 0), stop=(k == KC - 1))
            nc.vector.tensor_scalar(
                out=QT[:, hp, ds(CH * j, CH)], in0=ps[:],
                scalar1=bq_sb[:, hp : hp + 1], scalar2=None, op0=ADD)

        def v_tile(i):
            """V rows [128i : 128(i+1)] for all 8 heads (x chunk stationary)."""
            ps = psumP.tile([P, CH], f32, tag="pp", name=f"pv{i}")
            for k in range(KC):
                nc.tensor.matmul(ps[:], X[:, k, ts(i, P)], WV[:, k, :],
                                 start=(k == 0), stop=(k == KC - 1))
            nc.vector.tensor_copy(V[:, i, :, :],
                                  ps.rearrange("p (h d) -> p h d", d=D))

        def o_tile(j, m, ctxT_j):
            """Partial out rows [128m:...], q chunk j (no bias; host adds it)."""
            ps = psumP.tile([P, CH], f32, tag="pp", name=f"po{j}{m}")
            for k in range(OC):
                nc.tensor.matmul(ps[:], WO[:, k, ts(m, P)], ctxT_j[:, k, :],
                                 start=(k == 0), stop=(k == OC - 1))
            o_sb = opool.tile([P, CH], f32, tag="o", name=f"o{j}{m}")
            nc.vector.tensor_copy(o_sb[:], ps[:])
            nc.sync.dma_start(out[ts(m, P), ds(CH * j, CH)], o_sb[:])

        # ---------- attention ----------
        tri_b = tri_sb[:, 0:1, :].to_broadcast((P, 2, P))

        def attn_pair(hp, j, ctxT_j, fill):
            """Head pair hp, q rows [512j : 512(j+1)], kv in [0, 512(j+1))."""
            nkv = 4 * (j + 1)
            ctx_ps = psumC.tile([P, CH], f32, tag="ctx", name=f"cx{j}{hp}")
            acc = accpool.tile([P, 2, CH], bf16, tag="acc", name=f"ac{j}{hp}")
            pts = {}

            def score_step(c):
                m = c - 4 * j
                qo = P * m if m >= 0 else 0
                st = psumS.tile([P, 2, CH], f32, tag="st", name=f"st{j}{hp}{c}")
                for hh in range(2):
                    nc.tensor.matmul(
                        st[:, hh, qo:],
                        KT[ds(64 * hh, 64), hp, ts(c, P)],
                        QT[ds(64 * hh, 64), hp, ds(CH * j + qo, CH - qo)],
                        start=True, stop=True)
                pt = ptpool.tile([P, 2, CH], bf16, tag="pt", name=f"pt{j}{hp}{c}")
                nc.scalar.activation(pt[:, :, qo:], st[:, :, qo:], EXP)
                if m >= 0:
                    nc.vector.tensor_tensor(pt[:, :, ds(qo, P)],
                                            pt[:, :, ds(qo, P)], tri_b, MUL)
                if c == 0:
                    nc.vector.tensor_copy(acc[:], pt[:])
                else:
                    nc.vector.tensor_tensor(acc[:, :, qo:], acc[:, :, qo:],
                                            pt[:, :, qo:], ADD)
                pts[c] = (pt, qo)

            def av_step(c):
                pt, qo = pts.pop(c)
                for hh in range(2):
                    nc.tensor.matmul(
                        ctx_ps[ds(64 * hh, 64), qo:],
                        V[:, c, 2 * hp + hh, :],
                        pt[:, hh, qo:],
                        start=(c == 0), stop=(c == nkv - 1))

            # scores pipelined one step ahead of AV; fillers paced per step
            score_step(0)
            for c in range(1, nkv):
                score_step(c)
                av_step(c - 1)
                fill()
            av_step(nkv - 1)
            fill()

            # epilogue: l = colsum(P) via ones-matmul; ctxT = ctx / l
            l_ps = psumS.tile([P, 2, CH], f32, tag="st", name=f"l{j}{hp}")
            nc.tensor.matmul(l_ps[0:1, 0, :], ones_sb[:, 0:1], acc[:, 0, :],
                             start=True, stop=True)
            nc.tensor.matmul(l_ps[0:1, 1, :], ones_sb[:, 0:1], acc[:, 1, :],
                             start=True, stop=True)
            linv = lrpool.tile([1, 2, CH], f32, tag="lr", name=f"li{j}{hp}")
            nc.vector.reciprocal_approx_fast(linv[:, 0, :], l_ps[0:1, 0, :])
            nc.vector.reciprocal_approx_fast(linv[:, 1, :], l_ps[0:1, 1, :])
            for hh in range(2):
                lb = lbpool.tile([P, CH], f32, tag="lb", name=f"lb{j}{hp}{hh}")
                nc.gpsimd.partition_broadcast(lb[:], linv[:, hh, :],
                                              channels=P)
                nc.vector.tensor_tensor(ctxT_j[ds(64 * hh, 64), hp, :],
                                        ctx_ps[ds(64 * hh, 64), :],
                                        lb[ds(64 * hh, 64), :], MUL)

        # ---------- schedule ----------
        # prologue: K/V/Q for kv & q block 0
        for hp in range(NHP):
            kt_tile(0, hp)
        for i in range(NCH):
            v_tile(i)
        for hp in range(NHP):
            q_tile(0, hp)

        ctxT = [None] * NCH
        for j in range(NCH):
            ctxT[j] = ctxpool.tile([P, NHP, CH], bf16, tag="ctxT", name=f"cT{j}")
            # filler units due during attention chunk j
            units = []
            if j < NCH - 1:
                units += [lambda hp=hp: q_tile(j + 1, hp) for hp in range(NHP)]
                units += [lambda hp=hp: kt_tile(j + 1, hp) for hp in range(NHP)]
                units += [lambda i=i: v_tile(4 * (j + 1) + i) for i in range(4)]
            if j > 0:
                units += [lambda m=m, jj=j - 1: o_tile(jj, m, ctxT[jj])
                          for m in range(KC)]
            steps = NHP * 4 * (j + 1)
            state = {"s": 0, "f": 0}

            def fill(units=units, steps=steps, state=state):
                state["s"] += 1
                want = (len(units) * state["s"] + steps - 1) // steps
                while state["f"] < want and state["f"] < len(units):
                    units[state["f"]]()
                    state["f"] += 1

            for hp in range(NHP):
                attn_pair(hp, j, ctxT[j], fill)
            while state["f"] < len(units):
                units[state["f"]]()
                state["f"] += 1

        for m in range(KC):
            o_tile(NCH - 1, m, ctxT[NCH - 1])

    nc.compile()
    return nc


def _shard_inputs(x, Wq, bq, bk_, bv, bo, WqT, WkT, WvT, WoT):
    """Build the 8 per-core input maps. WqT is Wq.T/8; others are plain .T."""
    bf = ml_dtypes.bfloat16
    tri = np.triu(np.ones((P, P), np.float32)).astype(bf)
    in_maps = []
    for b in range(B):
        xTb = np.ascontiguousarray(x[b].T).astype(bf)
        for g in range(2):
            sl = slice(CH * g, CH * (g + 1))
            in_maps.append({
                "xT": xTb,
                "wqT": np.ascontiguousarray(WqT[:, sl]).astype(bf),
                "wkT": np.ascontiguousarray(WkT[:, sl]).astype(bf),
                "wvT": np.ascontiguousarray(WvT[:, sl]).astype(bf),
                "woT": np.ascontiguousarray(WoT[sl, :]).astype(bf),
                "bq": np.ascontiguousarray((bq[sl] / 8.0).reshape(NHP, P).T),
                "bk": np.ascontiguousarray(bk_[sl].reshape(NHP, P).T),
                "tri": tri,
            })
    return in_maps


def kernel(x, Wq, bq, Wk, bk, Wv, bv, Wo, bo):
    from concourse.bass_utils import run_bass_kernel_spmd

    x = np.asarray(x, np.float32)
    Wq = np.asarray(Wq, np.float32); bq = np.asarray(bq, np.float32)
    Wk = np.asarray(Wk, np.float32); bk = np.asarray(bk, np.float32)
    Wv = np.asarray(Wv, np.float32); bv = np.asarray(bv, np.float32)
    Wo = np.asarray(Wo, np.float32); bo = np.asarray(bo, np.float32)

    if "nc" not in _CACHE:
        _CACHE["nc"] = _build()
    nc = _CACHE["nc"]

    WqT = np.ascontiguousarray(Wq.T / 8.0)
    WkT = np.ascontiguousarray(Wk.T)
    WvT = np.ascontiguousarray(Wv.T)
    WoT = np.ascontiguousarray(Wo.T)
    in_maps = _shard_inputs(x, Wq, bq, bk, bv, bo, WqT, WkT, WvT, WoT)

    res = run_bass_kernel_spmd(nc, in_maps, core_ids=list(range(8)))
    bo_eff = (bo + Wo @ bv).astype(np.float32)
    outf = np.empty((B, T, C), np.float32)
    for b in range(B):
        o = res.results[2 * b]["out"] + res.results[2 * b + 1]["out"]  # (C, T)
        outf[b] = o.T + bo_eff
    return outf


# revision 10
# speedup vs baseline: 1.9097x; 1.0099x over previous
"""Causal multi-head attention (B=4, T=2048, C=1024, 16 heads) on 8 TRN2 NeuronCores.

Sharding: core (b, g) handles batch b and head-group g (8 heads = 512 features).
Each core projects Q/K/V for its own heads only (no duplicated projection work),
runs causal attention for its 8 heads over the full sequence, and computes a
PARTIAL output projection (contraction over its 512 ctx features). The host sums
the two partials per batch and adds the (bv-folded) output bias.

Causality: q-chunks of 512 attend to kv in [0, 512(j+1)); within the diagonal
512x512 block, score/exp/AV work is trimmed at 128 granularity and the
remaining triangular 128x128 blocks are masked with a single {0,1} tile.

Engine budget per core (bf16 matmuls, fp32 PSUM):
  PE  ~180us: QKVO projections (112) + scores/AV head-pair packed (60) + l-sums
  ACT ~163us: exp only (projection epilogues are on DVE instead)
  DVE ~110us: bias epilogues, P accumulation (for softmax denominators),
              masks, 1/l, ctx normalize, PSUM->SBUF copies
Softmax denominators: P tiles are accumulated on DVE into acc[kv,2,q]; a
ones-vector matmul reduces over kv partitions; normalize multiplies ctx by a
gpsimd-broadcast 1/l tile before the output projection.

Emission interleaves projection tiles as PE filler into the ACT-bound
attention stream (scores pipelined one step ahead of AV).
"""

import numpy as np
import ml_dtypes

B, T, C, NH, D = 4, 2048, 1024, 16, 64
P = 128
G = 8                 # heads per core
CH = 512              # q-chunk size
NCH = T // CH         # 4 q-chunks
KC = C // P           # 8 contraction chunks for QKV projections
OC = (C // 2) // P    # 4 contraction chunks for the partial O projection
NHP = G // 2          # 4 head pairs per core

_CACHE = {}


def _build():
    import concourse.bacc as bacc
    import concourse.tile as tile
    import concourse.mybir as mybir
    from concourse.bass import ts, ds

    f32 = mybir.dt.float32
    bf16 = mybir.dt.bfloat16
    EXP = mybir.ActivationFunctionType.Exp
    MUL = mybir.AluOpType.mult
    ADD = mybir.AluOpType.add

    nc = bacc.Bacc("TRN2", target_bir_lowering=False, debug=False, num_devices=8)

    def din(name, shape, dt=bf16):
        return nc.dram_tensor(name, list(shape), dt, kind="ExternalInput").ap()

    xT = din("xT", (C, T))           # x^T for this batch
    wqT = din("wqT", (C, CH))        # (Wq/8)^T columns for this head group
    wkT = din("wkT", (C, CH))
    wvT = din("wvT", (C, CH))
    woT = din("woT", (CH, C))        # Wo^T rows for this head group
    bq = din("bq", (P, NHP), f32)    # bq/8, chunked per 128-feature block
    bk = din("bk", (P, NHP), f32)
    tri = din("tri", (P, P))         # {0,1}, tri[kv, q] = kv <= q
    out = nc.dram_tensor("out", [C, T], bf16, kind="ExternalOutput").ap()

    x_v = xT.rearrange("(k p) t -> p k t", p=P)      # [128, 8, 2048]
    wq_v = wqT.rearrange("(k p) m -> p k m", p=P)    # [128, 8, 512]
    wk_v = wkT.rearrange("(k p) m -> p k m", p=P)
    wv_v = wvT.rearrange("(k p) m -> p k m", p=P)
    wo_v = woT.rearrange("(k p) m -> p k m", p=P)    # [128, 4, 1024]

    from contextlib import ExitStack
    with ExitStack() as ctx:
        tc = ctx.enter_context(tile.TileContext(nc))

        consts = ctx.enter_context(tc.tile_pool(name="consts", bufs=1))
        big = ctx.enter_context(tc.tile_pool(name="big", bufs=1))
        ctxpool = ctx.enter_context(tc.tile_pool(name="ctxT", bufs=2))
        ptpool = ctx.enter_context(tc.tile_pool(name="pt", bufs=4))
        accpool = ctx.enter_context(tc.tile_pool(name="acc", bufs=2))
        lrpool = ctx.enter_context(tc.tile_pool(name="lr", bufs=2))
        lbpool = ctx.enter_context(tc.tile_pool(name="lb", bufs=2))
        opool = ctx.enter_context(tc.tile_pool(name="o", bufs=2))
        psumS = ctx.enter_context(tc.tile_pool(name="psumS", bufs=2, space="PSUM"))
        psumC = ctx.enter_context(tc.tile_pool(name="psumC", bufs=2, space="PSUM"))
        psumP = ctx.enter_context(tc.tile_pool(name="psumP", bufs=2, space="PSUM"))

        bq_sb = consts.tile([P, NHP], f32)
        bk_sb = consts.tile([P, NHP], f32)
        tri_sb = consts.tile([P, 1, P], bf16)
        ones_sb = consts.tile([P, 1], bf16)
        warm = consts.tile([1, 2], f32)
        nc.vector.memset(ones_sb[:], 1.0)
        nc.vector.memset(warm[:], 0.0)
        # preload the exp table set early
        nc.scalar.activation(warm[:], warm[:], EXP)

        X = big.tile([P, KC, T], bf16)
        WQ = big.tile([P, KC, CH], bf16)
        WK = big.tile([P, KC, CH], bf16)
        WV = big.tile([P, KC, CH], bf16)
        WO = big.tile([P, OC, C], bf16)
        KT = big.tile([P, NHP, T], bf16)    # K^T  [d(2-head packed), hp, t]
        QT = big.tile([P, NHP, T], bf16)
        V = big.tile([P, T // P, G, D], bf16)  # [kv_local, kv_chunk, head, d]

        # DMA order = first-use order. X goes on the gpsimd queue, weights on
        # the sync queue (parallel); WK split per head-pair so the first
        # kt_tile can start after ~1/4 of it has landed.
        nc.gpsimd.dma_start(X[:, :, 0:CH], x_v[:, :, 0:CH])
        for hp in range(NHP):
            nc.sync.dma_start(WK[:, :, ts(hp, P)], wk_v[:, :, ts(hp, P)])
        nc.sync.dma_start(bk_sb[:], bk)
        nc.sync.dma_start(bq_sb[:], bq)
        nc.sync.dma_start(WV[:], wv_v)
        nc.sync.dma_start(WQ[:], wq_v)
        for tb in range(1, NCH):
            q = nc.gpsimd if tb == 2 else nc.sync
            q.dma_start(X[:, :, ds(CH * tb, CH)], x_v[:, :, ds(CH * tb, CH)])
        nc.sync.dma_start(tri_sb[:, 0, :], tri)
        nc.gpsimd.dma_start(WO[:], wo_v)

        # ---------- projection tile emitters (filler units) ----------
        def kt_tile(tb, hp):
            """KT[:, hp, 512*tb : ...] <- (Wk chunk)^T @ x chunk + bk."""
            ps = psumP.tile([P, CH], f32, tag="pp", name=f"pk{tb}{hp}")
            for k in range(KC):
                nc.tensor.matmul(ps[:], WK[:, k, ts(hp, P)],
                                 X[:, k, ds(CH * tb, CH)],
                                 start=(k == 0), stop=(k == KC - 1))
            nc.vector.tensor_scalar(
                out=KT[:, hp, ds(CH * tb, CH)], in0=ps[:],
                scalar1=bk_sb[:, hp : hp + 1], scalar2=None, op0=ADD)

        def q_tile(j, hp):
            ps = psumP.tile([P, CH], f32, tag="pp", name=f"pq{j}{hp}")
            for k in range(KC):
                nc.tensor.matmul(ps[:], WQ[:, k, ts(hp, P)],
                                 X[:, k, ds(CH * j, CH)],
                                 start=(k == 0), stop=(k == KC - 1))
            nc.vector.tensor_scalar(
                out=QT[:, hp, ds(CH * j, CH)], in0=ps[:],
                scalar1=bq_sb[:, hp : hp + 1], scalar2=None, op0=ADD)

        def v_tile(i):
            """V rows [128i : 128(i+1)] for all 8 heads (x chunk stationary)."""
            ps = psumP.tile([P, CH], f32, tag="pp", name=f"pv{i}")
            for k in range(KC):
                nc.tensor.matmul(ps[:], X[:, k, ts(i, P)], WV[:, k, :],
                                 start=(k == 0), stop=(k == KC - 1))
            nc.vector.tensor_copy(V[:, i, :, :],
                                  ps.rearrange("p (h d) -> p h d", d=D))

        def o_tile(j, m, ctxT_j):
            """Partial out rows [128m:...], q chunk j (no bias; host adds it)."""
            ps = psumP.tile([P, CH], f32, tag="pp", name=f"po{j}{m}")
            for k in range(OC):
                nc.tensor.matmul(ps[:], WO[:, k, ts(m, P)], ctxT_j[:, k, :],
                                 start=(k == 0), stop=(k == OC - 1))
            o_sb = opool.tile([P, CH], bf16, tag="o", name=f"o{j}{m}")
            nc.vector.tensor_copy(o_sb[:], ps[:])
            q = nc.gpsimd if m % 2 else nc.sync
            q.dma_start(out[ts(m, P), ds(CH * j, CH)], o_sb[:])

        # ---------- attention ----------
        tri_b = tri_sb[:, 0:1, :].to_broadcast((P, 2, P))

        def attn_pair(hp, j, ctxT_j, fill):
            """Head pair hp, q rows [512j : 512(j+1)], kv in [0, 512(j+1))."""
            nkv = 4 * (j + 1)
            ctx_ps = psumC.tile([P, CH], f32, tag="ctx", name=f"cx{j}{hp}")
            acc = accpool.tile([P, 2, CH], bf16, tag="acc", name=f"ac{j}{hp}")
            pts = {}

            def score_step(c):
                m = c - 4 * j
                qo = P * m if m >= 0 else 0
                st = psumS.tile([P, 2, CH], f32, tag="st", name=f"st{j}{hp}{c}")
                for hh in range(2):
                    nc.tensor.matmul(
                        st[:, hh, qo:],
                        KT[ds(64 * hh, 64), hp, ts(c, P)],
                        QT[ds(64 * hh, 64), hp, ds(CH * j + qo, CH - qo)],
                        start=True, stop=True)
                pt = ptpool.tile([P, 2, CH], bf16, tag="pt", name=f"pt{j}{hp}{c}")
                nc.scalar.activation(pt[:, :, qo:], st[:, :, qo:], EXP)
                if m >= 0:
                    nc.vector.tensor_tensor(pt[:, :, ds(qo, P)],
                                            pt[:, :, ds(qo, P)], tri_b, MUL)
                if c == 0:
                    nc.vector.tensor_copy(acc[:], pt[:])
                else:
                    nc.vector.tensor_tensor(acc[:, :, qo:], acc[:, :, qo:],
                                            pt[:, :, qo:], ADD)
                pts[c] = (pt, qo)

            def av_step(c):
                pt, qo = pts.pop(c)
                for hh in range(2):
                    nc.tensor.matmul(
                        ctx_ps[ds(64 * hh, 64), qo:],
                        V[:, c, 2 * hp + hh, :],
                        pt[:, hh, qo:],
                        start=(c == 0), stop=(c == nkv - 1))

            # scores pipelined one step ahead of AV; fillers paced per step
            # (emitted between the score and AV pairs so filler streaming
            # covers the AV LDWEIGHTS)
            score_step(0)
            for c in range(1, nkv):
                score_step(c)
                fill()
                av_step(c - 1)
            fill()
            av_step(nkv - 1)

            # epilogue: l = colsum(P) via ones-matmul; ctxT = ctx / l
            l_ps = psumS.tile([P, 2, CH], f32, tag="st", name=f"l{j}{hp}")
            nc.tensor.matmul(l_ps[0:1, 0, :], ones_sb[:, 0:1], acc[:, 0, :],
                             start=True, stop=True)
            nc.tensor.matmul(l_ps[0:1, 1, :], ones_sb[:, 0:1], acc[:, 1, :],
                             start=True, stop=True)
            linv = lrpool.tile([1, 2, CH], f32, tag="lr", name=f"li{j}{hp}")
            nc.vector.reciprocal_approx_fast(linv[:, 0, :], l_ps[0:1, 0, :])
            nc.vector.reciprocal_approx_fast(linv[:, 1, :], l_ps[0:1, 1, :])
            for hh in range(2):
                lb = lbpool.tile([P, CH], f32, tag="lb", name=f"lb{j}{hp}{hh}")
                nc.gpsimd.partition_broadcast(lb[:], linv[:, hh, :],
                                              channels=P)
                nc.vector.tensor_tensor(ctxT_j[ds(64 * hh, 64), hp, :],
                                        ctx_ps[ds(64 * hh, 64), :],
                                        lb[ds(64 * hh, 64), :], MUL)

        # ---------- schedule ----------
        # prologue: K/V/Q for kv & q block 0
        for hp in range(NHP):
            kt_tile(0, hp)
        for i in range(NCH):
            v_tile(i)
        for hp in range(NHP):
            q_tile(0, hp)

        ctxT = [None] * NCH
        for j in range(NCH):
            ctxT[j] = ctxpool.tile([P, NHP, CH], bf16, tag="ctxT", name=f"cT{j}")
            # filler units due during attention chunk j
            units = []
            if j < NCH - 1:
                units += [lambda hp=hp: q_tile(j + 1, hp) for hp in range(NHP)]
                units += [lambda hp=hp: kt_tile(j + 1, hp) for hp in range(NHP)]
                units += [lambda i=i: v_tile(4 * (j + 1) + i) for i in range(4)]
            if j > 0:
                units += [lambda m=m, jj=j - 1: o_tile(jj, m, ctxT[jj])
                          for m in range(KC)]
            steps = NHP * 4 * (j + 1)
            state = {"s": 0, "f": 0}

            def fill(units=units, steps=steps, state=state):
                state["s"] += 1
                want = (len(units) * state["s"] + steps - 1) // steps
                while state["f"] < want and state["f"] < len(units):
                    units[state["f"]]()
                    state["f"] += 1

            for hp in range(NHP):
                attn_pair(hp, j, ctxT[j], fill)
            while state["f"] < len(units):
                units[state["f"]]()
                state["f"] += 1

        for m in range(KC):
            o_tile(NCH - 1, m, ctxT[NCH - 1])

    nc.compile()
    return nc


def _shard_inputs(x, Wq, bq, bk_, bv, bo, WqT, WkT, WvT, WoT):
    """Build the 8 per-core input maps. WqT is Wq.T/8; others are plain .T."""
    bf = ml_dtypes.bfloat16
    tri = np.triu(np.ones((P, P), np.float32)).astype(bf)
    in_maps = []
    for b in range(B):
        xTb = np.ascontiguousarray(x[b].T).astype(bf)
        for g in range(2):
            sl = slice(CH * g, CH * (g + 1))
            in_maps.append({
                "xT": xTb,
                "wqT": np.ascontiguousarray(WqT[:, sl]).astype(bf),
                "wkT": np.ascontiguousarray(WkT[:, sl]).astype(bf),
                "wvT": np.ascontiguousarray(WvT[:, sl]).astype(bf),
                "woT": np.ascontiguousarray(WoT[sl, :]).astype(bf),
                "bq": np.ascontiguousarray((bq[sl] / 8.0).reshape(NHP, P).T),
                "bk": np.ascontiguousarray(bk_[sl].reshape(NHP, P).T),
                "tri": tri,
            })
    return in_maps


def kernel(x, Wq, bq, Wk, bk, Wv, bv, Wo, bo):
    from concourse.bass_utils import run_bass_kernel_spmd

    x = np.asarray(x, np.float32)
    Wq = np.asarray(Wq, np.float32); bq = np.asarray(bq, np.float32)
    Wk = np.asarray(Wk, np.float32); bk = np.asarray(bk, np.float32)
    Wv = np.asarray(Wv, np.float32); bv = np.asarray(bv, np.float32)
    Wo = np.asarray(Wo, np.float32); bo = np.asarray(bo, np.float32)

    if "nc" not in _CACHE:
        _CACHE["nc"] = _build()
    nc = _CACHE["nc"]

    WqT = np.ascontiguousarray(Wq.T / 8.0)
    WkT = np.ascontiguousarray(Wk.T)
    WvT = np.ascontiguousarray(Wv.T)
    WoT = np.ascontiguousarray(Wo.T)
    in_maps = _shard_inputs(x, Wq, bq, bk, bv, bo, WqT, WkT, WvT, WoT)

    res = run_bass_kernel_spmd(nc, in_maps, core_ids=list(range(8)))
    bo_eff = (bo + Wo @ bv).astype(np.float32)
    outf = np.empty((B, T, C), np.float32)
    for b in range(B):
        o = (res.results[2 * b]["out"].astype(np.float32)
             + res.results[2 * b + 1]["out"].astype(np.float32))  # (C, T)
        outf[b] = o.T + bo_eff
    return outf


# revision 26
# speedup vs baseline: 1.9674x; 1.0302x over previous
"""Causal multi-head attention (B=4, T=2048, C=1024, 16 heads) on 8 TRN2 NeuronCores.

Sharding: core (b, g) handles batch b and head-group g (8 heads = 512 features).
Each core projects Q/K/V for its own heads only (no duplicated projection work),
runs causal attention for its 8 heads over the full sequence, and computes a
PARTIAL output projection (contraction over its 512 ctx features). The host sums
the two partials per batch and adds the (bv-folded) output bias.

Causality: q-chunks of 512 attend to kv in [0, 512(j+1)); within the diagonal
512x512 block, score/exp/AV work is trimmed at 128 granularity and the
remaining triangular 128x128 blocks are masked with a single {0,1} tile.

Engine budget per core (bf16 matmuls, fp32 PSUM):
  PE  ~180us: QKVO projections (112) + scores/AV head-pair packed (60) + l-sums
  ACT ~163us: exp only (projection epilogues are on DVE instead)
  DVE ~110us: bias epilogues, P accumulation (for softmax denominators),
              masks, 1/l, ctx normalize, PSUM->SBUF copies
Softmax denominators: P tiles are accumulated on DVE into acc[kv,2,q]; a
ones-vector matmul reduces over kv partitions; normalize multiplies ctx by a
gpsimd-broadcast 1/l tile before the output projection.

Emission interleaves projection tiles as PE filler into the ACT-bound
attention stream (scores pipelined one step ahead of AV).
"""

import numpy as np
import ml_dtypes

B, T, C, NH, D = 4, 2048, 1024, 16, 64
P = 128
G = 8                 # heads per core
CH = 512              # q-chunk size
NCH = T // CH         # 4 q-chunks
KC = C // P           # 8 contraction chunks for QKV projections
OC = (C // 2) // P    # 4 contraction chunks for the partial O projection
NHP = G // 2          # 4 head pairs per core

_CACHE = {}


def _build():
    import concourse.bacc as bacc
    import concourse.tile as tile
    import concourse.mybir as mybir
    from concourse.bass import ts, ds

    f32 = mybir.dt.float32
    bf16 = mybir.dt.bfloat16
    f8 = mybir.dt.float8e4
    DR = mybir.MatmulPerfMode.DoubleRow
    EXP = mybir.ActivationFunctionType.Exp
    MUL = mybir.AluOpType.mult
    ADD = mybir.AluOpType.add

    nc = bacc.Bacc("TRN2", target_bir_lowering=False, debug=False, num_devices=8)

    def din(name, shape, dt=bf16):
        return nc.dram_tensor(name, list(shape), dt, kind="ExternalInput").ap()

    xT = din("xT", (C, T), f8)       # x^T for this batch (fp8: Q/K path only)
    xbT = din("xbT", (C, T))         # x^T in bf16 (V path: fp8 V noise does
                                     # not average out on short-context rows)
    wqT = din("wqT", (C, CH), f8)    # (Wq.T/8 * 64) columns for this head group
    wkT = din("wkT", (C, CH), f8)    # Wk.T * 64
    wvT = din("wvT", (C, CH))        # Wv.T (bf16)
    woT = din("woT", (CH, C))        # Wo.T rows for this head group (bf16)
    bq = din("bq", (P, NHP), f32)    # bq/8, chunked per 128-feature block
    bk = din("bk", (P, NHP), f32)
    tri = din("tri", (P, P))         # {0,1}, tri[kv, q] = kv <= q
    out = nc.dram_tensor("out", [C, T], bf16, kind="ExternalOutput").ap()

    x_v = xT.rearrange("(k p) t -> p k t", p=P)      # [128, 8, 2048]
    xb_v = xbT.rearrange("(k p) t -> p k t", p=P)
    wq_v = wqT.rearrange("(k p) m -> p k m", p=P)    # [128, 8, 512]
    wk_v = wkT.rearrange("(k p) m -> p k m", p=P)
    wv_v = wvT.rearrange("(k p) m -> p k m", p=P)
    wo_v = woT.rearrange("(k p) m -> p k m", p=P)    # [128, 4, 1024]

    from contextlib import ExitStack
    with ExitStack() as ctx:
        tc = ctx.enter_context(tile.TileContext(nc))

        consts = ctx.enter_context(tc.tile_pool(name="consts", bufs=1))
        big = ctx.enter_context(tc.tile_pool(name="big", bufs=1))
        ctxpool = ctx.enter_context(tc.tile_pool(name="ctxT", bufs=2))
        ptpool = ctx.enter_context(tc.tile_pool(name="pt", bufs=4))
        accpool = ctx.enter_context(tc.tile_pool(name="acc", bufs=2))
        lrpool = ctx.enter_context(tc.tile_pool(name="lr", bufs=2))
        lbpool = ctx.enter_context(tc.tile_pool(name="lb", bufs=2))
        opool = ctx.enter_context(tc.tile_pool(name="o", bufs=2))
        psumS = ctx.enter_context(tc.tile_pool(name="psumS", bufs=2, space="PSUM"))
        psumC = ctx.enter_context(tc.tile_pool(name="psumC", bufs=2, space="PSUM"))
        psumP = ctx.enter_context(tc.tile_pool(name="psumP", bufs=2, space="PSUM"))

        bq_sb = consts.tile([P, NHP], f32)
        bk_sb = consts.tile([P, NHP], f32)
        tri_sb = consts.tile([P, 1, P], bf16)
        ones_sb = consts.tile([P, 1], bf16)
        warm = consts.tile([1, 2], f32)
        nc.vector.memset(ones_sb[:], 1.0)
        nc.vector.memset(warm[:], 0.0)
        # preload the exp table set early
        nc.scalar.activation(warm[:], warm[:], EXP)

        X = big.tile([P, KC, T], f8)        # fp8 x (Q/K projections)
        XB = big.tile([P, KC, T], bf16)     # bf16 x (V projection)
        WQ = big.tile([P, KC, CH], f8)
        WK = big.tile([P, KC, CH], f8)
        WV = big.tile([P, KC, CH], bf16)
        WO = big.tile([P, OC, C], bf16)
        KT = big.tile([P, NHP, T], bf16)    # K^T  [d(2-head packed), hp, t]
        QT = big.tile([P, NHP, T], bf16)
        V = big.tile([P, T // P, G, D], bf16)  # [kv_local, kv_chunk, head, d]

        # DMA order = first-use order; X slice 0 + WK first (gate the first
        # matmul), weights on the sync queue, bulk X on gpsimd in parallel.
        nc.sync.dma_start(X[:, :, 0:CH], x_v[:, :, 0:CH])
        for hp in range(NHP):
            nc.sync.dma_start(WK[:, :, ts(hp, P)], wk_v[:, :, ts(hp, P)])
        nc.sync.dma_start(bk_sb[:], bk)
        nc.sync.dma_start(bq_sb[:], bq)
        nc.gpsimd.dma_start(XB[:, :, 0:CH], xb_v[:, :, 0:CH])
        nc.sync.dma_start(WV[:], wv_v)
        nc.sync.dma_start(WQ[:], wq_v)
        nc.gpsimd.dma_start(X[:, :, ds(CH, 3 * CH)], x_v[:, :, ds(CH, 3 * CH)])
        for tb in range(1, NCH):
            nc.sync.dma_start(XB[:, :, ds(CH * tb, CH)],
                              xb_v[:, :, ds(CH * tb, CH)])
        nc.sync.dma_start(tri_sb[:, 0, :], tri)
        nc.gpsimd.dma_start(WO[:], wo_v)

        # ---------- projection tile emitters (filler units) ----------
        # All projections are fp8 DoubleRow over k-chunk pairs; weights were
        # pre-scaled by 64 (32 for Wo) on the host, descaled in the epilogue.
        def kt_tile(tb, hp):
            """KT[:, hp, 512*tb : ...] <- (Wk chunk)^T @ x chunk + bk."""
            ps = psumP.tile([P, CH], f32, tag="pp", name=f"pk{tb}{hp}")
            for k in range(0, KC, 2):
                nc.tensor.matmul(ps[:], WK[:, k : k + 2, ts(hp, P)],
                                 X[:, k : k + 2, ds(CH * tb, CH)],
                                 start=(k == 0), stop=(k == KC - 2),
                                 perf_mode=DR)
            nc.vector.tensor_scalar(
                out=KT[:, hp, ds(CH * tb, CH)], in0=ps[:],
                scalar1=1.0 / 64, scalar2=bk_sb[:, hp : hp + 1],
                op0=MUL, op1=ADD)

        def q_tile(j, hp):
            ps = psumP.tile([P, CH], f32, tag="pp", name=f"pq{j}{hp}")
            for k in range(0, KC, 2):
                nc.tensor.matmul(ps[:], WQ[:, k : k + 2, ts(hp, P)],
                                 X[:, k : k + 2, ds(CH * j, CH)],
                                 start=(k == 0), stop=(k == KC - 2),
                                 perf_mode=DR)
            nc.vector.tensor_scalar(
                out=QT[:, hp, ds(CH * j, CH)], in0=ps[:],
                scalar1=1.0 / 64, scalar2=bq_sb[:, hp : hp + 1],
                op0=MUL, op1=ADD)

        def v_tile(i):
            """V rows [128i : 128(i+1)] for all 8 heads (x chunk stationary)."""
            ps = psumP.tile([P, CH], f32, tag="pp", name=f"pv{i}")
            for k in range(KC):
                nc.tensor.matmul(ps[:], XB[:, k, ts(i, P)], WV[:, k, :],
                                 start=(k == 0), stop=(k == KC - 1))
            nc.vector.tensor_copy(V[:, i, :, :],
                                  ps.rearrange("p (h d) -> p h d", d=D))

        def o_tile(j, m, ctxT_j):
            """Partial out rows [128m:...], q chunk j (no bias; host adds it)."""
            ps = psumP.tile([P, CH], f32, tag="pp", name=f"po{j}{m}")
            for k in range(OC):
                nc.tensor.matmul(ps[:], WO[:, k, ts(m, P)], ctxT_j[:, k, :],
                                 start=(k == 0), stop=(k == OC - 1))
            o_sb = opool.tile([P, CH], bf16, tag="o", name=f"o{j}{m}")
            nc.vector.tensor_copy(o_sb[:], ps[:])
            q = nc.gpsimd if m % 2 else nc.sync
            q.dma_start(out[ts(m, P), ds(CH * j, CH)], o_sb[:])

        # ---------- attention ----------
        tri_b = tri_sb[:, 0:1, :].to_broadcast((P, 2, P))

        def attn_pair(hp, j, ctxT_j, fill):
            """Head pair hp, q rows [512j : 512(j+1)], kv in [0, 512(j+1))."""
            nkv = 4 * (j + 1)
            ctx_ps = psumC.tile([P, CH], f32, tag="ctx", name=f"cx{j}{hp}")
            acc = accpool.tile([P, 2, CH], bf16, tag="acc", name=f"ac{j}{hp}")
            pts = {}

            def score_step(c):
                m = c - 4 * j
                qo = P * m if m >= 0 else 0
                st = psumS.tile([P, 2, CH], f32, tag="st", name=f"st{j}{hp}{c}")
                for hh in range(2):
                    nc.tensor.matmul(
                        st[:, hh, qo:],
                        KT[ds(64 * hh, 64), hp, ts(c, P)],
                        QT[ds(64 * hh, 64), hp, ds(CH * j + qo, CH - qo)],
                        start=True, stop=True)
                pt = ptpool.tile([P, 2, CH], bf16, tag="pt", name=f"pt{j}{hp}{c}")
                nc.scalar.activation(pt[:, :, qo:], st[:, :, qo:], EXP)
                if m >= 0:
                    nc.vector.tensor_tensor(pt[:, :, ds(qo, P)],
                                            pt[:, :, ds(qo, P)], tri_b, MUL)
                if c == 0:
                    nc.vector.tensor_copy(acc[:], pt[:])
                else:
                    nc.vector.tensor_tensor(acc[:, :, qo:], acc[:, :, qo:],
                                            pt[:, :, qo:], ADD)
                pts[c] = (pt, qo)

            def av_step(c):
                pt, qo = pts.pop(c)
                for hh in range(2):
                    nc.tensor.matmul(
                        ctx_ps[ds(64 * hh, 64), qo:],
                        V[:, c, 2 * hp + hh, :],
                        pt[:, hh, qo:],
                        start=(c == 0), stop=(c == nkv - 1))

            # scores pipelined one step ahead of AV; fillers paced per step
            # (emitted between the score and AV pairs so filler streaming
            # covers the AV LDWEIGHTS)
            score_step(0)
            for c in range(1, nkv):
                score_step(c)
                fill()
                av_step(c - 1)
            fill()
            av_step(nkv - 1)

            # epilogue: l = colsum(P) via ones-matmul; ctxT = ctx / l
            l_ps = psumS.tile([P, 2, CH], f32, tag="st", name=f"l{j}{hp}")
            nc.tensor.matmul(l_ps[0:1, 0, :], ones_sb[:, 0:1], acc[:, 0, :],
                             start=True, stop=True)
            nc.tensor.matmul(l_ps[0:1, 1, :], ones_sb[:, 0:1], acc[:, 1, :],
                             start=True, stop=True)
            linv = lrpool.tile([1, 2, CH], f32, tag="lr", name=f"li{j}{hp}")
            nc.vector.reciprocal_approx_fast(linv[:, 0, :], l_ps[0:1, 0, :])
            nc.vector.reciprocal_approx_fast(linv[:, 1, :], l_ps[0:1, 1, :])
            for hh in range(2):
                lb = lbpool.tile([P, CH], f32, tag="lb", name=f"lb{j}{hp}{hh}")
                nc.gpsimd.partition_broadcast(lb[:], linv[:, hh, :],
                                              channels=P)
                nc.vector.tensor_tensor(ctxT_j[ds(64 * hh, 64), hp, :],
                                        ctx_ps[ds(64 * hh, 64), :],
                                        lb[ds(64 * hh, 64), :], MUL)

        # ---------- schedule ----------
        # prologue: K/V/Q for kv & q block 0
        for hp in range(NHP):
            kt_tile(0, hp)
        for i in range(NCH):
            v_tile(i)
        for hp in range(NHP):
            q_tile(0, hp)

        ctxT = [None] * NCH
        for j in range(NCH):
            ctxT[j] = ctxpool.tile([P, NHP, CH], bf16, tag="ctxT", name=f"cT{j}")
            # filler units due during attention chunk j
            units = []
            if j < NCH - 1:
                units += [lambda hp=hp: q_tile(j + 1, hp) for hp in range(NHP)]
                units += [lambda hp=hp: kt_tile(j + 1, hp) for hp in range(NHP)]
                units += [lambda i=i: v_tile(4 * (j + 1) + i) for i in range(4)]
            if j > 0:
                units += [lambda m=m, jj=j - 1: o_tile(jj, m, ctxT[jj])
                          for m in range(KC)]
            steps = NHP * 4 * (j + 1)
            state = {"s": 0, "f": 0}

            def fill(units=units, steps=steps, state=state):
                state["s"] += 1
                want = (len(units) * state["s"] + steps - 1) // steps
                while state["f"] < want and state["f"] < len(units):
                    units[state["f"]]()
                    state["f"] += 1

            for hp in range(NHP):
                attn_pair(hp, j, ctxT[j], fill)
            while state["f"] < len(units):
                units[state["f"]]()
                state["f"] += 1

        for m in range(KC):
            o_tile(NCH - 1, m, ctxT[NCH - 1])

    nc.compile()
    return nc


def _shard_inputs(x, Wq, bq, bk_, bv, bo, WqT, WkT, WvT, WoT):
    """Build the 8 per-core input maps. WqT is Wq.T/8; others are plain .T.

    Data tensors go to the device in fp8e4m3; weights are pre-scaled by 64
    (32 for Wo) to clear the e4m3 denormal range, descaled on-device."""
    bf = ml_dtypes.bfloat16
    f8 = ml_dtypes.float8_e4m3
    tri = np.triu(np.ones((P, P), np.float32)).astype(bf)
    in_maps = []
    for b in range(B):
        xTb = np.ascontiguousarray(x[b].T)
        xT8 = xTb.astype(f8)
        xT16 = xTb.astype(bf)
        for g in range(2):
            sl = slice(CH * g, CH * (g + 1))
            in_maps.append({
                "xT": xT8,
                "xbT": xT16,
                "wqT": np.ascontiguousarray(WqT[:, sl] * 64.0).astype(f8),
                "wkT": np.ascontiguousarray(WkT[:, sl] * 64.0).astype(f8),
                "wvT": np.ascontiguousarray(WvT[:, sl]).astype(bf),
                "woT": np.ascontiguousarray(WoT[sl, :]).astype(bf),
                "bq": np.ascontiguousarray((bq[sl] / 8.0).reshape(NHP, P).T),
                "bk": np.ascontiguousarray(bk_[sl].reshape(NHP, P).T),
                "tri": tri,
            })
    return in_maps


def kernel(x, Wq, bq, Wk, bk, Wv, bv, Wo, bo):
    from concourse.bass_utils import run_bass_kernel_spmd

    x = np.asarray(x, np.float32)
    Wq = np.asarray(Wq, np.float32); bq = np.asarray(bq, np.float32)
    Wk = np.asarray(Wk, np.float32); bk = np.asarray(bk, np.float32)
    Wv = np.asarray(Wv, np.float32); bv = np.asarray(bv, np.float32)
    Wo = np.asarray(Wo, np.float32); bo = np.asarray(bo, np.float32)

    if "nc" not in _CACHE:
        _CACHE["nc"] = _build()
    nc = _CACHE["nc"]

    WqT = np.ascontiguousarray(Wq.T / 8.0)
    WkT = np.ascontiguousarray(Wk.T)
    WvT = np.ascontiguousarray(Wv.T)
    WoT = np.ascontiguousarray(Wo.T)
    in_maps = _shard_inputs(x, Wq, bq, bk, bv, bo, WqT, WkT, WvT, WoT)

    res = run_bass_kernel_spmd(nc, in_maps, core_ids=list(range(8)))
    bo_eff = (bo + Wo @ bv).astype(np.float32)
    outf = np.empty((B, T, C), np.float32)
    for b in range(B):
        o = (res.results[2 * b]["out"].astype(np.float32)
             + res.results[2 * b + 1]["out"].astype(np.float32))  # (C, T)
        outf[b] = o.T + bo_eff
    return outf
